# revision 6
# baseline (speedup 1.0000x reference)
"""Trainium2 Bass kernel for nn_Attention (dense transformer attention block).

Full causal attention: QKV projection + RoPE + softmax(QK^T/sqrt(d) + mask)V + WO,
bsz=1, seqlen=2048, dim=4096, 32 heads x head_dim 128, fp32 I/O.

Sharding: tensor-parallel across heads on 8 NeuronCores. Core c owns heads
4c..4c+3 (wq/wk/wv output columns, attention) and wo output columns
512c..512c+512 (after an AllGather of the per-core attn^T shard along the
head axis). Host concatenates the 8 output column shards.

All matmuls run as float32r (fp32 storage, reduced-precision single-pass PE
mode, ~1e-4 relative accuracy at full PE rate).
"""

import numpy as np

import concourse.bacc as bacc
import concourse.mybir as mybir
import concourse.tile as tile
from concourse.bass_utils import run_bass_kernel_spmd

# Problem constants (hardcoded per contract)
N_CORES = 8
S = 2048              # sequence length
D = 4096              # model dim
HD = 128              # head dim
NH_LOC = 4            # heads per core
DSH = 512             # per-core shard width (NH_LOC * HD)
KT = D // 128         # 32 contraction tiles over model dim
QTILES = S // 128     # 16 token tiles
QRANGES = S // 512    # 4 query ranges of 512
SCALE = float(1.0 / np.sqrt(HD))

F32 = mybir.dt.float32
F32R = mybir.dt.float32r

_PROGRAMS = {}


def _build_program(mode, debug_stop=None):
    """mode: 'causal' (triu -1e9 mask), 'nomask' (zero mask), 'general'
    (arbitrary additive mask streamed from DRAM)."""
    causal = mode == "causal"
    general = mode == "general"
    do_attn = debug_stop not in ("qkv",)
    do_ag = do_attn and debug_stop not in ("attn",)
    do_wo = do_ag and debug_stop not in ("ag",)

    nc = bacc.Bacc("TRN2", target_bir_lowering=False, debug=False,
                   num_devices=N_CORES)

    # ---- external inputs (per core) ----
    xT_d = nc.dram_tensor("xT", [D, S], F32R, kind="ExternalInput")
    wq_d = nc.dram_tensor("wq", [D, DSH], F32R, kind="ExternalInput")
    wk_d = nc.dram_tensor("wk", [D, DSH], F32R, kind="ExternalInput")
    wv_d = nc.dram_tensor("wv", [D, DSH], F32R, kind="ExternalInput")
    wo_d = nc.dram_tensor("wo", [D, DSH], F32R, kind="ExternalInput")
    fr_d = nc.dram_tensor("fr128", [128, S], F32, kind="ExternalInput")
    fis_d = nc.dram_tensor("fis128", [128, S], F32, kind="ExternalInput")
    perm_d = nc.dram_tensor("perm", [128, 128], F32R, kind="ExternalInput")
    ones128_d = nc.dram_tensor("ones128", [128, 1], F32R, kind="ExternalInput")
    ones1_d = nc.dram_tensor("ones1", [1, 128], F32R, kind="ExternalInput")
    if causal:
        maskt_d = nc.dram_tensor("maskt", [128, 128], F32, kind="ExternalInput")
    if general:
        masktf_d = nc.dram_tensor("masktf", [S, S], F32, kind="ExternalInput")
    out_d = nc.dram_tensor("out", [S, DSH], F32, kind="ExternalOutput")
    dbg_d = None
    if debug_stop == "qkv":
        dbg_d = nc.dram_tensor("dbg", [2 * DSH, S], F32, kind="ExternalOutput")
    if debug_stop in ("attn", "ag"):
        dbg_d = nc.dram_tensor("dbg", [DSH, S], F32, kind="ExternalOutput")

    with tile.TileContext(nc) as tc:
        with (
            tc.tile_pool(name="consts", bufs=1) as cns,
            tc.tile_pool(name="dram", bufs=1, space="DRAM") as dram,
        ):
            # resident constants
            fr_sb = cns.tile([128, S], F32, tag="fr")
            fis_sb = cns.tile([128, S], F32, tag="fis")
            perm_sb = cns.tile([128, 128], F32R, tag="perm")
            ones128_sb = cns.tile([128, 1], F32R, tag="o128")
            ones1_sb = cns.tile([1, 128], F32R, tag="o1")
            nc.sync.dma_start(fr_sb[:], fr_d[:, :])
            nc.sync.dma_start(fis_sb[:], fis_d[:, :])
            nc.sync.dma_start(perm_sb[:], perm_d[:, :])
            nc.sync.dma_start(ones128_sb[:], ones128_d[:, :])
            nc.sync.dma_start(ones1_sb[:], ones1_d[:, :])
            if causal:
                maskt_sb = cns.tile([128, 128], F32, tag="maskt")
                nc.sync.dma_start(maskt_sb[:], maskt_d[:, :])

            # DRAM spill / bounce tensors
            qt_spill = dram.tile([DSH, S], F32R)    # Q^T rotated, [d, s]
            kt_spill = dram.tile([DSH, S], F32R)    # K^T rotated, [d, s]
            v_spill = dram.tile([S, DSH], F32R)     # V, [s, d]
            ag_in = dram.tile([DSH, S], F32)        # attn^T shard
            ag_out = dram.tile([D, S], F32, addr_space="Shared")

            # ================= Phase 1: QKV projections =================
            for half in range(2):
                s0 = half * 1024
                with tc.tile_pool(name=f"xt{half}", bufs=1) as xtp:
                    xt_sb = xtp.tile([128, KT, 1024], F32R, tag="xt")
                    # load x^T half in 4 chunks so early matmuls start sooner
                    for ch in range(4):
                        nc.sync.dma_start(
                            xt_sb[:, ch * 8:(ch + 1) * 8, :],
                            xT_d[ch * 1024:(ch + 1) * 1024, s0:s0 + 1024]
                            .rearrange("(kt p) s -> p kt s", p=128),
                        )

                    # ---- Q^T and K^T (with RoPE) ----
                    with (
                        tc.tile_pool(name=f"qkw{half}", bufs=2) as qkw,
                        tc.tile_pool(name=f"qkd{half}", bufs=3) as qkd,
                        tc.tile_pool(name=f"qkp{half}", bufs=2, space="PSUM") as qkp,
                    ):
                        for oi in range(8):
                            w_src = wq_d if oi < 4 else wk_d
                            spill = qt_spill if oi < 4 else kt_spill
                            head = oi % 4
                            psums = [qkp.tile([128, 512], F32, tag=f"qk{j}",
                                              name=f"qkps{j}")
                                     for j in range(2)]
                            for wc in range(4):
                                w_c = qkw.tile([128, 8, 128], F32R, tag="w")
                                nc.sync.dma_start(
                                    w_c[:],
                                    w_src[wc * 1024:(wc + 1) * 1024,
                                          head * 128:(head + 1) * 128]
                                    .rearrange("(kt p) m -> p kt m", p=128),
                                )
                                for kt8 in range(8):
                                    kt = wc * 8 + kt8
                                    for j in range(2):
                                        nc.tensor.matmul(
                                            psums[j][:],
                                            w_c[:, kt8, :],
                                            xt_sb[:, kt, j * 512:(j + 1) * 512],
                                            start=(kt == 0), stop=(kt == KT - 1),
                                        )
                            for j in range(2):
                                qrange = half * 2 + j
                                qt_sb = qkd.tile([128, 512], F32R, tag="qt")
                                nc.scalar.copy(qt_sb[:], psums[j][:])
                                swap_ps = qkp.tile([128, 512], F32, tag="swap")
                                nc.tensor.matmul(swap_ps[:], perm_sb[:], qt_sb[:])
                                tmp1 = qkd.tile([128, 512], F32, tag="t1")
                                nc.vector.tensor_mul(
                                    tmp1[:], qt_sb[:],
                                    fr_sb[:, qrange * 512:(qrange + 1) * 512])
                                tmp2 = qkd.tile([128, 512], F32, tag="t2")
                                nc.vector.tensor_mul(
                                    tmp2[:], swap_ps[:],
                                    fis_sb[:, qrange * 512:(qrange + 1) * 512])
                                rot = qkd.tile([128, 512], F32R, tag="rot")
                                nc.vector.tensor_add(rot[:], tmp1[:], tmp2[:])
                                nc.sync.dma_start(
                                    spill[head * 128:(head + 1) * 128,
                                          qrange * 512:(qrange + 1) * 512],
                                    rot[:],
                                )

                    # ---- V ----
                    with (
                        tc.tile_pool(name=f"vw{half}", bufs=2) as vw,
                        tc.tile_pool(name=f"vd{half}", bufs=3) as vd,
                        tc.tile_pool(name=f"vp{half}", bufs=8, space="PSUM") as vp,
                    ):
                        psv = [vp.tile([128, 512], F32, tag=f"vps{t}",
                                       name=f"vps{t}", bufs=1)
                               for t in range(8)]
                        for ktc in range(8):
                            wv_c = vw.tile([128, 4, 512], F32R, tag="wv")
                            nc.sync.dma_start(
                                wv_c[:],
                                wv_d[ktc * 512:(ktc + 1) * 512, :]
                                .rearrange("(kt p) n -> p kt n", p=128),
                            )
                            for kt4 in range(4):
                                kt = ktc * 4 + kt4
                                for tt in range(8):
                                    nc.tensor.matmul(
                                        psv[tt][:],
                                        xt_sb[:, kt, tt * 128:(tt + 1) * 128],
                                        wv_c[:, kt4, :],
                                        start=(kt == 0), stop=(kt == KT - 1),
                                    )
                        for tt in range(8):
                            v_sb = vd.tile([128, 512], F32R, tag="vsb")
                            nc.scalar.copy(v_sb[:], psv[tt][:])
                            nc.sync.dma_start(
                                v_spill[(half * 8 + tt) * 128:
                                        (half * 8 + tt + 1) * 128, :],
                                v_sb[:],
                            )

            # ================= Phase 2: attention =================
            if debug_stop == "qkv":
                # dump spills for inspection: out <- v_spill (same shape)
                nc.sync.dma_start(out_d[:, :], v_spill[:, :].bitcast(F32))
                nc.sync.dma_start(dbg_d[0:DSH, :], qt_spill[:, :].bitcast(F32))
                nc.sync.dma_start(dbg_d[DSH:2 * DSH, :], kt_spill[:, :].bitcast(F32))
            if do_attn:
              with (
                tc.tile_pool(name="akv", bufs=2) as akv,
                tc.tile_pool(name="aq", bufs=2) as aq,
                tc.tile_pool(name="apt", bufs=20) as apt,
                tc.tile_pool(name="awk", bufs=3) as awk,
                tc.tile_pool(name="aps", bufs=2, space="PSUM") as aps,
              ):
                for head in range(NH_LOC):
                    kt_h = akv.tile([128, S], F32R, tag="kth")
                    nc.sync.dma_start(
                        kt_h[:], kt_spill[head * 128:(head + 1) * 128, :])
                    v_h = akv.tile([128, QTILES, 128], F32R, tag="vh")
                    nc.sync.dma_start(
                        v_h[:],
                        v_spill[:, head * 128:(head + 1) * 128]
                        .rearrange("(tt p) d -> p tt d", p=128),
                    )
                    for qr in range(QRANGES):
                        q_sb = aq.tile([128, 512], F32R, tag="qsb")
                        nc.sync.dma_start(
                            q_sb[:],
                            qt_spill[head * 128:(head + 1) * 128,
                                     qr * 512:(qr + 1) * 512],
                        )
                        nkt = (4 * qr + 4) if causal else QTILES
                        pts = []
                        for kt in range(nkt):
                            ps_t = aps.tile([128, 512], F32, tag="st")
                            nc.tensor.matmul(
                                ps_t[:],
                                kt_h[:, kt * 128:(kt + 1) * 128],
                                q_sb[:],
                            )
                            pT = apt.tile([128, 512], F32R, tag="pT")
                            if general:
                                mt = awk.tile([128, 512], F32, tag="mt")
                                nc.sync.dma_start(
                                    mt[:],
                                    masktf_d[kt * 128:(kt + 1) * 128,
                                             qr * 512:(qr + 1) * 512])
                                msk = awk.tile([128, 512], F32, tag="msk")
                                nc.vector.scalar_tensor_tensor(
                                    msk[:], ps_t[:], SCALE, mt[:],
                                    op0=mybir.AluOpType.mult,
                                    op1=mybir.AluOpType.add)
                                nc.scalar.activation(
                                    pT[:], msk[:],
                                    mybir.ActivationFunctionType.Exp)
                            elif not causal or kt < 4 * qr:
                                # fully unmasked block
                                nc.scalar.activation(
                                    pT[:], ps_t[:],
                                    mybir.ActivationFunctionType.Exp,
                                    scale=SCALE)
                            else:
                                # block row kt intersects the diagonal
                                for qtl in range(4):
                                    qtile = qr * 4 + qtl
                                    blk = slice(qtl * 128, (qtl + 1) * 128)
                                    if qtile < kt:
                                        nc.vector.tensor_scalar_mul(
                                            pT[:, blk], ps_t[:, blk], 0.0)
                                    elif qtile == kt:
                                        msk = awk.tile([128, 128], F32,
                                                       tag="mskd")
                                        nc.vector.scalar_tensor_tensor(
                                            msk[:], ps_t[:, blk], SCALE,
                                            maskt_sb[:],
                                            op0=mybir.AluOpType.mult,
                                            op1=mybir.AluOpType.add)
                                        nc.scalar.activation(
                                            pT[:, blk], msk[:],
                                            mybir.ActivationFunctionType.Exp)
                                    else:
                                        nc.scalar.activation(
                                            pT[:, blk], ps_t[:, blk],
                                            mybir.ActivationFunctionType.Exp,
                                            scale=SCALE)
                            pts.append(pT)

                        ps_pv = aps.tile([128, 512], F32, tag="pv")
                        for kt in range(nkt):
                            nc.tensor.matmul(
                                ps_pv[:], v_h[:, kt, :], pts[kt][:],
                                start=(kt == 0), stop=(kt == nkt - 1))
                        ps_rs = aps.tile([1, 512], F32, tag="rs")
                        for kt in range(nkt):
                            nc.tensor.matmul(
                                ps_rs[:], ones128_sb[:], pts[kt][:],
                                start=(kt == 0), stop=(kt == nkt - 1))
                        rrow = awk.tile([1, 512], F32R, tag="rrow")
                        with nc.allow_low_precision(reason="recip rounded to f32r for PE broadcast"):
                            nc.vector.reciprocal(rrow[:], ps_rs[:])
                        ps_bc = aps.tile([128, 512], F32, tag="bc")
                        nc.tensor.matmul(ps_bc[:], ones1_sb[:], rrow[:])
                        bc_sb = awk.tile([128, 512], F32, tag="bcs")
                        nc.scalar.copy(bc_sb[:], ps_bc[:])
                        at_sb = awk.tile([128, 512], F32R, tag="at")
                        nc.vector.tensor_mul(at_sb[:], ps_pv[:], bc_sb[:])
                        nc.sync.dma_start(
                            ag_in[head * 128:(head + 1) * 128,
                                  qr * 512:(qr + 1) * 512],
                            at_sb[:].bitcast(F32),
                        )

            # ================= Phase 3: AllGather attn^T =================
            if debug_stop == "attn":
                nc.sync.dma_start(dbg_d[:, :], ag_in[:, :])
            if do_ag:
              nc.gpsimd.collective_compute(
                "AllGather",
                mybir.AluOpType.bypass,
                replica_groups=[list(range(N_CORES))],
                ins=[ag_in[:].opt()],
                outs=[ag_out[:].opt()],
              )

            # ================= Phase 4: output projection =================
            if debug_stop == "ag":
                nc.sync.dma_start(dbg_d[:, :], ag_out[0:DSH, :])
            if do_wo:
              with (
                tc.tile_pool(name="wop", bufs=1) as wop,
                tc.tile_pool(name="woa", bufs=2) as woa,
                tc.tile_pool(name="woo", bufs=3) as woo,
                tc.tile_pool(name="wops", bufs=2, space="PSUM") as wops,
              ):
                wo_sb = wop.tile([128, KT, DSH], F32R, tag="wo")
                for ch in range(4):
                    nc.sync.dma_start(
                        wo_sb[:, ch * 8:(ch + 1) * 8, :],
                        wo_d[ch * 1024:(ch + 1) * 1024, :]
                        .rearrange("(dt p) c -> p dt c", p=128),
                    )
                for qt in range(QTILES):
                    atq = woa.tile([128, KT, 128], F32R, tag="atq")
                    nc.sync.dma_start(
                        atq[:],
                        ag_out[:, qt * 128:(qt + 1) * 128]
                        .rearrange("(dt p) q -> p dt q", p=128).bitcast(F32R),
                    )
                    ps_o = wops.tile([128, 512], F32, tag="wops")
                    for dt in range(KT):
                        nc.tensor.matmul(
                            ps_o[:], atq[:, dt, :], wo_sb[:, dt, :],
                            start=(dt == 0), stop=(dt == KT - 1))
                    o_sb = woo.tile([128, 512], F32, tag="osb")
                    nc.scalar.copy(o_sb[:], ps_o[:])
                    nc.sync.dma_start(
                        out_d[qt * 128:(qt + 1) * 128, :], o_sb[:])

    nc.compile()
    return nc


def _get_program(mode, debug_stop=None):
    key = (mode, debug_stop)
    if key not in _PROGRAMS:
        _PROGRAMS[key] = _build_program(mode, debug_stop)
    return _PROGRAMS[key]


def _prep_inputs(x, wq, wk, wv, wo, freqs_real, freqs_imag, mask):
    """Host-side shard/layout prep. Returns (mode, in_maps)."""
    x = np.asarray(x, dtype=np.float32)
    wq = np.asarray(wq, dtype=np.float32)
    wk = np.asarray(wk, dtype=np.float32)
    wv = np.asarray(wv, dtype=np.float32)
    wo = np.asarray(wo, dtype=np.float32)
    fr = np.asarray(freqs_real, dtype=np.float32)
    fi = np.asarray(freqs_imag, dtype=np.float32)
    m = np.asarray(mask, dtype=np.float32).reshape(S, S)

    causal_ref = np.triu(np.full((S, S), np.float32(-1e9), dtype=np.float32), k=1)
    if np.array_equal(m, causal_ref):
        mode = "causal"
    elif not m.any():
        mode = "nomask"
    else:
        mode = "general"

    xT = np.ascontiguousarray(x.reshape(S, D).T)  # [D, S]

    # evens-first permutation of each head's 128 dims (for RoPE pair layout)
    idx = np.concatenate([np.arange(0, HD, 2), np.arange(1, HD, 2)])
    cols = np.concatenate([h * HD + idx for h in range(32)])
    wq_p = wq[:, cols]
    wk_p = wk[:, cols]

    fr128 = np.ascontiguousarray(np.concatenate([fr.T, fr.T], axis=0))   # [128, S]
    fis128 = np.ascontiguousarray(np.concatenate([-fi.T, fi.T], axis=0))

    perm = np.zeros((128, 128), dtype=np.float32)
    perm[np.arange(128), (np.arange(128) + 64) % 128] = 1.0

    ones128 = np.ones((128, 1), dtype=np.float32)
    ones1 = np.ones((1, 128), dtype=np.float32)

    in_maps = []
    for c in range(N_CORES):
        sl = slice(c * DSH, (c + 1) * DSH)
        im = {
            "xT": xT,
            "wq": np.ascontiguousarray(wq_p[:, sl]),
            "wk": np.ascontiguousarray(wk_p[:, sl]),
            "wv": np.ascontiguousarray(wv[:, sl]),
            "wo": np.ascontiguousarray(wo[:, sl]),
            "fr128": fr128,
            "fis128": fis128,
            "perm": perm,
            "ones128": ones128,
            "ones1": ones1,
        }
        if mode == "causal":
            # mask tile in [k, q] layout: valid iff k <= q
            maskt = np.where(
                np.arange(128)[:, None] <= np.arange(128)[None, :],
                np.float32(0.0), np.float32(-1e9)).astype(np.float32)
            im["maskt"] = maskt
        if mode == "general":
            im["masktf"] = np.ascontiguousarray(m.T)
        in_maps.append(im)
    return mode, in_maps


def kernel(x, wq, wk, wv, wo, cache_k, cache_v, freqs_real, freqs_imag,
           mask, start_pos, **_unused):
    assert int(start_pos) == 0, "kernel hardcodes start_pos=0"
    mode, in_maps = _prep_inputs(x, wq, wk, wv, wo, freqs_real, freqs_imag, mask)
    nc = _get_program(mode)
    res = run_bass_kernel_spmd(nc, in_maps, core_ids=list(range(N_CORES)))
    out = np.concatenate([res.results[c]["out"] for c in range(N_CORES)], axis=1)
    return out.reshape(1, S, D).astype(np.float32)


# revision 7
# speedup vs baseline: 1.0614x; 1.0614x over previous
"""Trainium2 Bass kernel for nn_Attention (dense transformer attention block).

Full causal attention: QKV projection + RoPE + softmax(QK^T/sqrt(d) + mask)V + WO,
bsz=1, seqlen=2048, dim=4096, 32 heads x head_dim 128, fp32 I/O.

Sharding: tensor-parallel across heads on 8 NeuronCores. Core c owns heads
4c..4c+3 (wq/wk/wv output columns, attention) and wo output columns
512c..512c+512 (after an AllGather of the per-core attn^T shard along the
head axis). Host concatenates the 8 output column shards.

All matmuls run as float32r (fp32 storage, reduced-precision single-pass PE
mode, ~1e-4 relative accuracy at full PE rate).
"""

import numpy as np

import concourse.bacc as bacc
import concourse.mybir as mybir
import concourse.tile as tile
from concourse.bass_utils import run_bass_kernel_spmd

# Problem constants (hardcoded per contract)
N_CORES = 8
S = 2048              # sequence length
D = 4096              # model dim
HD = 128              # head dim
NH_LOC = 4            # heads per core
DSH = 512             # per-core shard width (NH_LOC * HD)
KT = D // 128         # 32 contraction tiles over model dim
QTILES = S // 128     # 16 token tiles
QRANGES = S // 512    # 4 query ranges of 512
SCALE = float(1.0 / np.sqrt(HD))

F32 = mybir.dt.float32
F32R = mybir.dt.float32r

_PROGRAMS = {}


def _build_program(mode, debug_stop=None):
    """mode: 'causal' (triu -1e9 mask), 'nomask' (zero mask), 'general'
    (arbitrary additive mask streamed from DRAM)."""
    causal = mode == "causal"
    general = mode == "general"
    do_attn = debug_stop not in ("qkv",)
    do_ag = do_attn and debug_stop not in ("attn",)
    do_wo = do_ag and debug_stop not in ("ag",)

    nc = bacc.Bacc("TRN2", target_bir_lowering=False, debug=False,
                   num_devices=N_CORES)

    # ---- external inputs (per core) ----
    xT_d = nc.dram_tensor("xT", [D, S], F32R, kind="ExternalInput")
    wq_d = nc.dram_tensor("wq", [D, DSH], F32R, kind="ExternalInput")
    wk_d = nc.dram_tensor("wk", [D, DSH], F32R, kind="ExternalInput")
    wv_d = nc.dram_tensor("wv", [D, DSH], F32R, kind="ExternalInput")
    wo_d = nc.dram_tensor("wo", [D, DSH], F32R, kind="ExternalInput")
    fr_d = nc.dram_tensor("fr128", [128, S], F32, kind="ExternalInput")
    fis_d = nc.dram_tensor("fis128", [128, S], F32, kind="ExternalInput")
    perm_d = nc.dram_tensor("perm", [128, 128], F32R, kind="ExternalInput")
    ones128_d = nc.dram_tensor("ones128", [128, 1], F32R, kind="ExternalInput")
    ones1_d = nc.dram_tensor("ones1", [1, 128], F32R, kind="ExternalInput")
    if causal:
        maskt_d = nc.dram_tensor("maskt", [128, 128], F32, kind="ExternalInput")
    if general:
        masktf_d = nc.dram_tensor("masktf", [S, S], F32, kind="ExternalInput")
    out_d = nc.dram_tensor("out", [S, DSH], F32, kind="ExternalOutput")
    dbg_d = None
    if debug_stop == "qkv":
        dbg_d = nc.dram_tensor("dbg", [2 * DSH, S], F32, kind="ExternalOutput")
    if debug_stop in ("attn", "ag"):
        dbg_d = nc.dram_tensor("dbg", [DSH, S], F32, kind="ExternalOutput")

    with tile.TileContext(nc) as tc:
        with (
            tc.tile_pool(name="consts", bufs=1) as cns,
            tc.tile_pool(name="dram", bufs=1, space="DRAM") as dram,
        ):
            # DRAM spill / bounce tensors
            qt_spill = dram.tile([DSH, S], F32R)    # Q^T rotated, [d, s]
            kt_spill = dram.tile([DSH, S], F32R)    # K^T rotated, [d, s]
            v_spill = dram.tile([S, DSH], F32R)     # V, [s, d]
            agi = [dram.tile([DSH, 512], F32, name=f"agi{r}") for r in range(4)]
            ago = [dram.tile([D, 512], F32, addr_space="Shared", name=f"ago{r}")
                   for r in range(4)]

            # small resident constants
            ones128_sb = cns.tile([128, 1], F32R, tag="o128")
            ones1_sb = cns.tile([1, 128], F32R, tag="o1")
            if causal:
                maskt_sb = cns.tile([128, 128], F32, tag="maskt")

            # ================= Phase 1: QKV projections =================
            with (
                tc.tile_pool(name="p1c", bufs=1) as p1c,
                tc.tile_pool(name="xtp", bufs=4) as xtp,
            ):
                perm_sb = p1c.tile([128, 128], F32R, tag="perm")
                fr_sb = p1c.tile([128, S], F32, tag="fr")
                fis_sb = p1c.tile([128, S], F32, tag="fis")

                first = True
                for half in range(2):
                    s0 = half * 1024
                    # x^T quarters: slot cycling (bufs=4) lets half-1 quarters
                    # prefetch as half-0 quarters free up
                    xts = []
                    for q in range(4):
                        xq = xtp.tile([128, 8, 1024], F32R, tag="xt",
                                      name=f"xt_{half}_{q}")
                        nc.sync.dma_start(
                            xq[:],
                            xT_d[q * 1024:(q + 1) * 1024, s0:s0 + 1024]
                            .rearrange("(kt p) s -> p kt s", p=128),
                        )
                        xts.append(xq)
                        if first:
                            # interleave const loads after the first xt chunk
                            nc.scalar.dma_start(perm_sb[:], perm_d[:, :])
                            nc.scalar.dma_start(fr_sb[:], fr_d[:, :])
                            nc.scalar.dma_start(fis_sb[:], fis_d[:, :])
                            nc.scalar.dma_start(ones128_sb[:], ones128_d[:, :])
                            nc.scalar.dma_start(ones1_sb[:], ones1_d[:, :])
                            if causal:
                                nc.scalar.dma_start(maskt_sb[:], maskt_d[:, :])
                            first = False

                    # ---- Q^T and K^T (with RoPE) ----
                    with (
                        tc.tile_pool(name=f"qkw{half}", bufs=2) as qkw,
                        tc.tile_pool(name=f"qkd{half}", bufs=3) as qkd,
                        tc.tile_pool(name=f"qkp{half}", bufs=2, space="PSUM") as qkp,
                    ):
                        for oi in range(8):
                            w_src = wq_d if oi < 4 else wk_d
                            spill = qt_spill if oi < 4 else kt_spill
                            head = oi % 4
                            psums = [qkp.tile([128, 512], F32, tag=f"qk{j}",
                                              name=f"qkps{j}")
                                     for j in range(2)]
                            for wc in range(4):
                                w_c = qkw.tile([128, 8, 128], F32R, tag="w")
                                nc.scalar.dma_start(
                                    w_c[:],
                                    w_src[wc * 1024:(wc + 1) * 1024,
                                          head * 128:(head + 1) * 128]
                                    .rearrange("(kt p) m -> p kt m", p=128),
                                )
                                for kt8 in range(8):
                                    kt = wc * 8 + kt8
                                    for j in range(2):
                                        nc.tensor.matmul(
                                            psums[j][:],
                                            w_c[:, kt8, :],
                                            xts[wc][:, kt8, j * 512:(j + 1) * 512],
                                            start=(kt == 0), stop=(kt == KT - 1),
                                        )
                            for j in range(2):
                                qrange = half * 2 + j
                                qt_sb = qkd.tile([128, 512], F32R, tag="qt")
                                nc.scalar.copy(qt_sb[:], psums[j][:])
                                swap_ps = qkp.tile([128, 512], F32, tag="swap")
                                nc.tensor.matmul(swap_ps[:], perm_sb[:], qt_sb[:])
                                tmp1 = qkd.tile([128, 512], F32, tag="t1")
                                nc.vector.tensor_mul(
                                    tmp1[:], qt_sb[:],
                                    fr_sb[:, qrange * 512:(qrange + 1) * 512])
                                tmp2 = qkd.tile([128, 512], F32, tag="t2")
                                nc.vector.tensor_mul(
                                    tmp2[:], swap_ps[:],
                                    fis_sb[:, qrange * 512:(qrange + 1) * 512])
                                rot = qkd.tile([128, 512], F32R, tag="rot")
                                nc.vector.tensor_add(rot[:], tmp1[:], tmp2[:])
                                nc.sync.dma_start(
                                    spill[head * 128:(head + 1) * 128,
                                          qrange * 512:(qrange + 1) * 512],
                                    rot[:],
                                )

                    # ---- V ----
                    with (
                        tc.tile_pool(name=f"vw{half}", bufs=2) as vw,
                        tc.tile_pool(name=f"vd{half}", bufs=3) as vd,
                        tc.tile_pool(name=f"vp{half}", bufs=8, space="PSUM") as vp,
                    ):
                        psv = [vp.tile([128, 512], F32, tag=f"vps{t}",
                                       name=f"vps{t}", bufs=1)
                               for t in range(8)]
                        for ktc in range(8):
                            wv_c = vw.tile([128, 4, 512], F32R, tag="wv")
                            nc.scalar.dma_start(
                                wv_c[:],
                                wv_d[ktc * 512:(ktc + 1) * 512, :]
                                .rearrange("(kt p) n -> p kt n", p=128),
                            )
                            for kt4 in range(4):
                                kt = ktc * 4 + kt4
                                for tt in range(8):
                                    nc.tensor.matmul(
                                        psv[tt][:],
                                        xts[kt // 8][:, kt % 8,
                                                     tt * 128:(tt + 1) * 128],
                                        wv_c[:, kt4, :],
                                        start=(kt == 0), stop=(kt == KT - 1),
                                    )
                        for tt in range(8):
                            v_sb = vd.tile([128, 512], F32R, tag="vsb")
                            nc.scalar.copy(v_sb[:], psv[tt][:])
                            nc.sync.dma_start(
                                v_spill[(half * 8 + tt) * 128:
                                        (half * 8 + tt + 1) * 128, :],
                                v_sb[:],
                            )

            # ========== Phase 2+3+4: attention / AllGather / WO, interleaved ==========
            if debug_stop == "qkv":
                nc.sync.dma_start(out_d[:, :], v_spill[:, :].bitcast(F32))
                nc.sync.dma_start(dbg_d[0:DSH, :], qt_spill[:, :].bitcast(F32))
                nc.sync.dma_start(dbg_d[DSH:2 * DSH, :], kt_spill[:, :].bitcast(F32))
            if do_attn:
                with (
                    tc.tile_pool(name="akv", bufs=1) as akv,
                    tc.tile_pool(name="aq", bufs=2) as aq,
                    tc.tile_pool(name="apt", bufs=16) as apt,
                    tc.tile_pool(name="awk", bufs=2) as awk,
                    tc.tile_pool(name="wop", bufs=1) as wop,
                    tc.tile_pool(name="woa", bufs=2) as woa,
                    tc.tile_pool(name="woo", bufs=2) as woo,
                    tc.tile_pool(name="aps", bufs=1, space="PSUM") as aps,
                ):
                    # resident K^T and V for all local heads
                    kts, vhs = [], []
                    for head in range(NH_LOC):
                        kt_h = akv.tile([128, S], F32R, tag=f"kth{head}",
                                        name=f"kth{head}")
                        nc.sync.dma_start(
                            kt_h[:], kt_spill[head * 128:(head + 1) * 128, :])
                        kts.append(kt_h)
                        v_h = akv.tile([128, QTILES, 128], F32R,
                                       tag=f"vh{head}", name=f"vh{head}")
                        nc.sync.dma_start(
                            v_h[:],
                            v_spill[:, head * 128:(head + 1) * 128]
                            .rearrange("(tt p) d -> p tt d", p=128),
                        )
                        vhs.append(v_h)
                    # WO weights resident (loads overlap attention)
                    wo_sb = wop.tile([128, KT, DSH], F32R, tag="wo")
                    for ch in range(4):
                        nc.scalar.dma_start(
                            wo_sb[:, ch * 8:(ch + 1) * 8, :],
                            wo_d[ch * 1024:(ch + 1) * 1024, :]
                            .rearrange("(dt p) c -> p dt c", p=128),
                        )

                    def wo_qrange(r):
                        """WO matmuls for the 4 output qtiles of qrange r."""
                        for qtl in range(4):
                            qt = r * 4 + qtl
                            ps_o = aps.tile([128, 512], F32, tag="wops",
                                            name=f"wops{qt}", bufs=2)
                            for hh in range(2):
                                atq = woa.tile([128, 16, 128], F32R, tag="atq",
                                               name=f"atq{qt}_{hh}")
                                nc.sync.dma_start(
                                    atq[:],
                                    ago[r][hh * 2048:(hh + 1) * 2048,
                                           qtl * 128:(qtl + 1) * 128]
                                    .rearrange("(dt p) q -> p dt q", p=128)
                                    .bitcast(F32R),
                                )
                                for dt in range(16):
                                    gdt = hh * 16 + dt
                                    nc.tensor.matmul(
                                        ps_o[:], atq[:, dt, :],
                                        wo_sb[:, gdt, :],
                                        start=(gdt == 0), stop=(gdt == KT - 1))
                            o_sb = woo.tile([128, 512], F32, tag="osb",
                                            name=f"osb{qt}")
                            nc.scalar.copy(o_sb[:], ps_o[:])
                            nc.sync.dma_start(
                                out_d[qt * 128:(qt + 1) * 128, :], o_sb[:])

                    for qr in range(QRANGES):
                        for head in range(NH_LOC):
                            kt_h = kts[head]
                            v_h = vhs[head]
                            q_sb = aq.tile([128, 512], F32R, tag="qsb",
                                           name=f"qsb{qr}_{head}")
                            nc.sync.dma_start(
                                q_sb[:],
                                qt_spill[head * 128:(head + 1) * 128,
                                         qr * 512:(qr + 1) * 512],
                            )
                            nkt = (4 * qr + 4) if causal else QTILES
                            pts = []
                            for kt in range(nkt):
                                ps_t = aps.tile([128, 512], F32, tag="st",
                                                name=f"st{qr}_{head}_{kt}",
                                                bufs=2)
                                nc.tensor.matmul(
                                    ps_t[:],
                                    kt_h[:, kt * 128:(kt + 1) * 128],
                                    q_sb[:],
                                )
                                pT = apt.tile([128, 512], F32R, tag="pT",
                                              name=f"pT{qr}_{head}_{kt}")
                                if general:
                                    mt = awk.tile([128, 512], F32, tag="mt")
                                    nc.sync.dma_start(
                                        mt[:],
                                        masktf_d[kt * 128:(kt + 1) * 128,
                                                 qr * 512:(qr + 1) * 512])
                                    msk = awk.tile([128, 512], F32, tag="msk")
                                    nc.vector.scalar_tensor_tensor(
                                        msk[:], ps_t[:], SCALE, mt[:],
                                        op0=mybir.AluOpType.mult,
                                        op1=mybir.AluOpType.add)
                                    nc.scalar.activation(
                                        pT[:], msk[:],
                                        mybir.ActivationFunctionType.Exp)
                                elif not causal or kt < 4 * qr:
                                    nc.scalar.activation(
                                        pT[:], ps_t[:],
                                        mybir.ActivationFunctionType.Exp,
                                        scale=SCALE)
                                else:
                                    for qtl in range(4):
                                        qtile = qr * 4 + qtl
                                        blk = slice(qtl * 128, (qtl + 1) * 128)
                                        if qtile < kt:
                                            nc.vector.tensor_scalar_mul(
                                                pT[:, blk], ps_t[:, blk], 0.0)
                                        elif qtile == kt:
                                            msk = awk.tile([128, 128], F32,
                                                           tag="mskd")
                                            nc.vector.scalar_tensor_tensor(
                                                msk[:], ps_t[:, blk], SCALE,
                                                maskt_sb[:],
                                                op0=mybir.AluOpType.mult,
                                                op1=mybir.AluOpType.add)
                                            nc.scalar.activation(
                                                pT[:, blk], msk[:],
                                                mybir.ActivationFunctionType.Exp)
                                        else:
                                            nc.scalar.activation(
                                                pT[:, blk], ps_t[:, blk],
                                                mybir.ActivationFunctionType.Exp,
                                                scale=SCALE)
                                pts.append(pT)

                            ps_pv = aps.tile([128, 512], F32, tag="pv",
                                             name=f"pv{qr}_{head}", bufs=2)
                            for kt in range(nkt):
                                nc.tensor.matmul(
                                    ps_pv[:], v_h[:, kt, :], pts[kt][:],
                                    start=(kt == 0), stop=(kt == nkt - 1))
                            ps_rs = aps.tile([1, 512], F32, tag="rs",
                                             name=f"rs{qr}_{head}", bufs=1)
                            for kt in range(nkt):
                                nc.tensor.matmul(
                                    ps_rs[:], ones128_sb[:], pts[kt][:],
                                    start=(kt == 0), stop=(kt == nkt - 1))
                            rrow = awk.tile([1, 512], F32R, tag="rrow")
                            with nc.allow_low_precision(reason="f32r recip"):
                                nc.vector.reciprocal(rrow[:], ps_rs[:])
                            ps_bc = aps.tile([128, 512], F32, tag="bc",
                                             name=f"bc{qr}_{head}", bufs=1)
                            nc.tensor.matmul(ps_bc[:], ones1_sb[:], rrow[:])
                            bc_sb = awk.tile([128, 512], F32, tag="bcs")
                            nc.scalar.copy(bc_sb[:], ps_bc[:])
                            at_sb = awk.tile([128, 512], F32R, tag="at")
                            nc.vector.tensor_mul(at_sb[:], ps_pv[:], bc_sb[:])
                            nc.sync.dma_start(
                                agi[qr][head * 128:(head + 1) * 128, :],
                                at_sb[:].bitcast(F32),
                            )
                        # AllGather this qrange's attn^T slice
                        if do_ag:
                            nc.gpsimd.collective_compute(
                                "AllGather",
                                mybir.AluOpType.bypass,
                                replica_groups=[list(range(N_CORES))],
                                ins=[agi[qr][:].opt()],
                                outs=[ago[qr][:].opt()],
                            )
                            if do_wo and qr > 0:
                                wo_qrange(qr - 1)
                    if do_wo:
                        wo_qrange(QRANGES - 1)
                    if debug_stop == "attn":
                        for r in range(4):
                            nc.sync.dma_start(
                                dbg_d[:, r * 512:(r + 1) * 512], agi[r][:, :])
                    if debug_stop == "ag":
                        for r in range(4):
                            nc.sync.dma_start(
                                dbg_d[:, r * 512:(r + 1) * 512],
                                ago[r][0:DSH, :])

    nc.compile()
    return nc


def _get_program(mode, debug_stop=None):
    key = (mode, debug_stop)
    if key not in _PROGRAMS:
        _PROGRAMS[key] = _build_program(mode, debug_stop)
    return _PROGRAMS[key]


def _prep_inputs(x, wq, wk, wv, wo, freqs_real, freqs_imag, mask):
    """Host-side shard/layout prep. Returns (mode, in_maps)."""
    x = np.asarray(x, dtype=np.float32)
    wq = np.asarray(wq, dtype=np.float32)
    wk = np.asarray(wk, dtype=np.float32)
    wv = np.asarray(wv, dtype=np.float32)
    wo = np.asarray(wo, dtype=np.float32)
    fr = np.asarray(freqs_real, dtype=np.float32)
    fi = np.asarray(freqs_imag, dtype=np.float32)
    m = np.asarray(mask, dtype=np.float32).reshape(S, S)

    causal_ref = np.triu(np.full((S, S), np.float32(-1e9), dtype=np.float32), k=1)
    if np.array_equal(m, causal_ref):
        mode = "causal"
    elif not m.any():
        mode = "nomask"
    else:
        mode = "general"

    xT = np.ascontiguousarray(x.reshape(S, D).T)  # [D, S]

    # evens-first permutation of each head's 128 dims (for RoPE pair layout)
    idx = np.concatenate([np.arange(0, HD, 2), np.arange(1, HD, 2)])
    cols = np.concatenate([h * HD + idx for h in range(32)])
    wq_p = wq[:, cols]
    wk_p = wk[:, cols]

    fr128 = np.ascontiguousarray(np.concatenate([fr.T, fr.T], axis=0))   # [128, S]
    fis128 = np.ascontiguousarray(np.concatenate([-fi.T, fi.T], axis=0))

    perm = np.zeros((128, 128), dtype=np.float32)
    perm[np.arange(128), (np.arange(128) + 64) % 128] = 1.0

    ones128 = np.ones((128, 1), dtype=np.float32)
    ones1 = np.ones((1, 128), dtype=np.float32)

    in_maps = []
    for c in range(N_CORES):
        sl = slice(c * DSH, (c + 1) * DSH)
        im = {
            "xT": xT,
            "wq": np.ascontiguousarray(wq_p[:, sl]),
            "wk": np.ascontiguousarray(wk_p[:, sl]),
            "wv": np.ascontiguousarray(wv[:, sl]),
            "wo": np.ascontiguousarray(wo[:, sl]),
            "fr128": fr128,
            "fis128": fis128,
            "perm": perm,
            "ones128": ones128,
            "ones1": ones1,
        }
        if mode == "causal":
            # mask tile in [k, q] layout: valid iff k <= q
            maskt = np.where(
                np.arange(128)[:, None] <= np.arange(128)[None, :],
                np.float32(0.0), np.float32(-1e9)).astype(np.float32)
            im["maskt"] = maskt
        if mode == "general":
            im["masktf"] = np.ascontiguousarray(m.T)
        in_maps.append(im)
    return mode, in_maps


def kernel(x, wq, wk, wv, wo, cache_k, cache_v, freqs_real, freqs_imag,
           mask, start_pos, **_unused):
    assert int(start_pos) == 0, "kernel hardcodes start_pos=0"
    mode, in_maps = _prep_inputs(x, wq, wk, wv, wo, freqs_real, freqs_imag, mask)
    nc = _get_program(mode)
    res = run_bass_kernel_spmd(nc, in_maps, core_ids=list(range(N_CORES)))
    out = np.concatenate([res.results[c]["out"] for c in range(N_CORES)], axis=1)
    return out.reshape(1, S, D).astype(np.float32)
